# revision 11
# baseline (speedup 1.0000x reference)
"""Trainium2 Bass kernel for EnhancedCrossAttention3D.

Computes, per batch b:
    q = Wq @ x1 + bq            (x1 = branch1[b] reshaped [C, N])
    k = Wk @ x2 + bk
    v = Wv @ x2 + bv
    attn = softmax((q^T k) / sqrt(C), axis=keys)
    out = Wp @ (attn @ v^T)^T + bp      -> [C, N]

Sharding: 8 cores = 2 batches x 4 query shards of 2048. Each core gets its
full K/V source (branch2[b]) and its query shard; no collectives.

On-core algorithm (flash-style, S^T layout):
    S^T[m, n] = sum_c k[c, m] * qT[c, n]   (m = key index on partitions)
    E = exp(S^T / 8)                       (logits are tiny; no max-sub needed)
    PV[c, n]  = sum_m [v | 1][m, c] * E[m, n]   -> row 64 is the softmax denom
    out[o, n] = (Wp @ PV[0:64]) / denom + (Wp @ bv + bp)
(bv is folded in after normalization: attn rows sum to 1.)
"""

import numpy as np
from contextlib import ExitStack

import concourse.bass as bass
import concourse.mybir as mybir
import concourse.tile as tile
from concourse import bacc
from concourse.bass import ts
from concourse.bass_utils import run_bass_kernel_spmd

B, C, D, H, W = 2, 64, 8, 32, 32
N = D * H * W              # 8192 keys per batch
NCORES = 8
QSH = (B * N) // NCORES    # 2048 queries per core
MCH = N // 128             # 64 key chunks of 128
NT = QSH // 512            # 4 query tiles of 512
F32 = mybir.dt.float32
F32R = mybir.dt.float32r
AF = mybir.ActivationFunctionType

_CACHE = {}


def _emit(tc, xq, xkv, wq, wk, wv, wp, bq, bk, bv, bp, out):
    nc = tc.nc
    ctx = ExitStack()
    # float32r is bit-identical to float32 storage; it only selects the PE's
    # full-rate fp32 streaming mode, so these writes lose no precision.
    ctx.enter_context(nc.allow_low_precision(reason="float32r == float32 bits"))
    const = ctx.enter_context(tc.tile_pool(name="const", bufs=1))
    big = ctx.enter_context(tc.tile_pool(name="big", bufs=1))
    ps = ctx.enter_context(tc.tile_pool(name="ps", bufs=2, space="PSUM"))
    ps_acc_p = ctx.enter_context(tc.tile_pool(name="ps_acc", bufs=1, space="PSUM"))
    ex_pool = ctx.enter_context(tc.tile_pool(name="ex", bufs=3))
    small = ctx.enter_context(tc.tile_pool(name="small", bufs=4))

    # ---- loads ----
    xq_sb = big.tile([C, QSH], F32R)
    nc.sync.dma_start(out=xq_sb, in_=xq)
    xkv_sb = big.tile([C, N], F32R)
    nc.sync.dma_start(out=xkv_sb, in_=xkv)
    wqT = const.tile([C, C], F32R)
    nc.sync.dma_start(out=wqT, in_=wq.rearrange("o c -> c o"))
    wkT = const.tile([C, C], F32R)
    nc.sync.dma_start(out=wkT, in_=wk.rearrange("o c -> c o"))
    wvT = const.tile([C, C], F32R)
    nc.sync.dma_start(out=wvT, in_=wv.rearrange("o c -> c o"))
    wpT = const.tile([C, C], F32R)
    nc.sync.dma_start(out=wpT, in_=wp.rearrange("o c -> c o"))
    bq_sb = const.tile([C, 1], F32)
    nc.sync.dma_start(out=bq_sb, in_=bq.rearrange("(c one) -> c one", one=1))
    bk_sb = const.tile([C, 1], F32)
    nc.sync.dma_start(out=bk_sb, in_=bk.rearrange("(c one) -> c one", one=1))
    bv_sb = const.tile([C, 1], F32)
    nc.sync.dma_start(out=bv_sb, in_=bv.rearrange("(c one) -> c one", one=1))
    bp_sb = const.tile([C, 1], F32)
    nc.sync.dma_start(out=bp_sb, in_=bp.rearrange("(c one) -> c one", one=1))
    # memset can't target f32r (it bitcasts internally); stage ones in f32
    ones_f32 = const.tile([128, MCH], F32)
    nc.vector.memset(ones_f32, 1.0)

    # ---- projections ----
    # qT[o, n] on partitions o
    qT_sb = big.tile([C, QSH], F32R)
    for t in range(NT):
        pq = ps.tile([128, 1024], F32, tag="ps")
        nc.tensor.matmul(pq[0:C, 0:512], lhsT=wqT, rhs=xq_sb[:, ts(t, 512)],
                         start=True, stop=True)
        nc.vector.tensor_scalar_add(qT_sb[:, ts(t, 512)], pq[0:C, 0:512], bq_sb)
    # k[o, m] on partitions o
    k_sb = big.tile([C, N], F32R)
    for t in range(N // 512):
        pk = ps.tile([128, 1024], F32, tag="ps")
        nc.tensor.matmul(pk[0:C, 0:512], lhsT=wkT, rhs=xkv_sb[:, ts(t, 512)],
                         start=True, stop=True)
        nc.vector.tensor_scalar_add(k_sb[:, ts(t, 512)], pk[0:C, 0:512], bk_sb)
    # v[m, c] chunks of 128 keys, with an extra ones column (denominator)
    v_sb = big.tile([128, MCH, C + 1], F32R)
    nc.vector.tensor_copy(v_sb[:, :, C], ones_f32)
    for i in range(MCH):
        pv = ps.tile([128, 1024], F32, tag="ps")
        nc.tensor.matmul(pv[:, 0:C], lhsT=xkv_sb[:, ts(i, 128)], rhs=wvT,
                         start=True, stop=True)
        nc.vector.tensor_copy(v_sb[:, i, 0:C], pv[:, 0:C])

    # ---- flash loop over key chunks ----
    ps_acc = ps_acc_p.tile([C + 1, QSH], F32)
    for i in range(MCH):
        for h in range(2):
            s_ps = ps.tile([128, 1024], F32, tag="ps")
            for sub in range(2):
                nt = h * 2 + sub
                nc.tensor.matmul(s_ps[:, ts(sub, 512)],
                                 lhsT=k_sb[:, ts(i, 128)],
                                 rhs=qT_sb[:, ts(nt, 512)],
                                 start=True, stop=True)
            ex = ex_pool.tile([128, 1024], F32R)
            nc.scalar.activation(ex, s_ps, AF.Exp, scale=0.125)
            for sub in range(2):
                nt = h * 2 + sub
                nc.tensor.matmul(ps_acc[:, ts(nt, 512)],
                                 lhsT=v_sb[:, i, :],
                                 rhs=ex[:, ts(sub, 512)],
                                 start=(i == 0), stop=(i == MCH - 1),
                                 skip_group_check=True)

    # ---- epilogue: normalize (+bv), project, +bp ----
    recip = small.tile([1, QSH], F32, tag="recip")
    nc.vector.reciprocal(recip, ps_acc[C:C + 1, :])
    rb = small.tile([C, QSH], F32, tag="rb")
    nc.gpsimd.partition_broadcast(rb, recip)
    for t in range(NT):
        pvn = small.tile([C, 512], F32R, tag="pvn")
        nc.vector.tensor_mul(pvn, ps_acc[0:C, ts(t, 512)], rb[:, ts(t, 512)])
        nc.vector.tensor_scalar_add(pvn, pvn, bv_sb)
        pp = ps.tile([128, 1024], F32, tag="ps")
        nc.tensor.matmul(pp[0:C, 0:512], lhsT=wpT, rhs=pvn,
                         start=True, stop=True)
        o_sb = small.tile([C, 512], F32, tag="o")
        nc.scalar.activation(o_sb, pp[0:C, 0:512], AF.Identity, bias=bp_sb,
                             scale=1.0)
        nc.sync.dma_start(out=out[:, ts(t, 512)], in_=o_sb)
    ctx.close()


def _build():
    nc = bacc.Bacc("TRN2", target_bir_lowering=False, debug=False,
                   num_devices=NCORES)
    aps = {}
    aps["xq"] = nc.dram_tensor("xq", [C, QSH], F32R, kind="ExternalInput").ap()
    aps["xkv"] = nc.dram_tensor("xkv", [C, N], F32R, kind="ExternalInput").ap()
    for nm in ("wq", "wk", "wv", "wp"):
        aps[nm] = nc.dram_tensor(nm, [C, C], F32R, kind="ExternalInput").ap()
    for nm in ("bq", "bk", "bv", "bp"):
        aps[nm] = nc.dram_tensor(nm, [C], F32, kind="ExternalInput").ap()
    aps["out"] = nc.dram_tensor("out", [C, QSH], F32, kind="ExternalOutput").ap()
    with tile.TileContext(nc) as tc:
        _emit(tc, **aps)
    nc.finalize()
    return nc


def kernel(branch1, branch2, Wq, bq, Wk, bk, Wv, bv, Wp, bp, **run_kwargs):
    if "nc" not in _CACHE:
        _CACHE["nc"] = _build()
    nc = _CACHE["nc"]

    x1 = np.ascontiguousarray(np.asarray(branch1, np.float32).reshape(B, C, N))
    x2 = np.ascontiguousarray(np.asarray(branch2, np.float32).reshape(B, C, N))
    consts = {
        "wq": np.ascontiguousarray(Wq, np.float32),
        "wk": np.ascontiguousarray(Wk, np.float32),
        "wv": np.ascontiguousarray(Wv, np.float32),
        "wp": np.ascontiguousarray(Wp, np.float32),
        "bq": np.ascontiguousarray(bq, np.float32),
        "bk": np.ascontiguousarray(bk, np.float32),
        "bv": np.ascontiguousarray(bv, np.float32),
        "bp": np.ascontiguousarray(bp, np.float32),
    }
    in_maps = []
    for core in range(NCORES):
        b, s = divmod(core, NCORES // B)
        in_maps.append({
            "xq": np.ascontiguousarray(x1[b, :, s * QSH:(s + 1) * QSH]),
            "xkv": x2[b],
            **consts,
        })
    res = run_bass_kernel_spmd(nc, in_maps, core_ids=list(range(NCORES)),
                               **run_kwargs)
    out = np.empty((B, C, N), np.float32)
    for core in range(NCORES):
        b, s = divmod(core, NCORES // B)
        out[b, :, s * QSH:(s + 1) * QSH] = res.results[core]["out"]
    if run_kwargs:
        _CACHE["last_result"] = res
    return out.reshape(B, C, D, H, W)


# revision 13
# speedup vs baseline: 1.0891x; 1.0891x over previous
"""Trainium2 Bass kernel for EnhancedCrossAttention3D.

Computes, per batch b:
    q = Wq @ x1 + bq            (x1 = branch1[b] reshaped [C, N])
    k = Wk @ x2 + bk
    v = Wv @ x2 + bv
    attn = softmax((q^T k) / sqrt(C), axis=keys)
    out = Wp @ (attn @ v^T)^T + bp      -> [C, N]

Sharding: 8 cores = 2 batches x 4 query shards of 2048. Each core gets its
full K/V source (branch2[b]) and its query shard; no collectives.

On-core algorithm (flash-style, S^T layout):
    S^T[m, n] = sum_c k[c, m] * qT[c, n]   (m = key index on partitions)
    E = exp(S^T / 8)                       (logits are tiny; no max-sub needed)
    PV[c, n]  = sum_m [v | 1][m, c] * E[m, n]   -> row 64 is the softmax denom
    out[o, n] = (Wp @ PV[0:64]) / denom + (Wp @ bv + bp)
(bv is folded in after normalization: attn rows sum to 1.)
"""

import numpy as np
from contextlib import ExitStack

import concourse.bass as bass
import concourse.mybir as mybir
import concourse.tile as tile
from concourse import bacc
from concourse.bass import ts
from concourse.bass_utils import run_bass_kernel_spmd

B, C, D, H, W = 2, 64, 8, 32, 32
N = D * H * W              # 8192 keys per batch
NCORES = 8
QSH = (B * N) // NCORES    # 2048 queries per core
MCH = N // 128             # 64 key chunks of 128
NT = QSH // 512            # 4 query tiles of 512
F32 = mybir.dt.float32
F32R = mybir.dt.float32r
BF16 = mybir.dt.bfloat16
AF = mybir.ActivationFunctionType

_CACHE = {}


def _emit(tc, xq, xkv, wq, wk, wv, wp, bq, bk, bv, bp, out):
    nc = tc.nc
    ctx = ExitStack()
    # float32r is bit-identical to float32 storage; it only selects the PE's
    # full-rate fp32 streaming mode, so these writes lose no precision.
    ctx.enter_context(nc.allow_low_precision(reason="float32r == float32 bits"))
    const = ctx.enter_context(tc.tile_pool(name="const", bufs=1))
    big = ctx.enter_context(tc.tile_pool(name="big", bufs=1))
    ps = ctx.enter_context(tc.tile_pool(name="ps", bufs=2, space="PSUM"))
    ps_acc_p = ctx.enter_context(tc.tile_pool(name="ps_acc", bufs=1, space="PSUM"))
    ex_pool = ctx.enter_context(tc.tile_pool(name="ex", bufs=3))
    small = ctx.enter_context(tc.tile_pool(name="small", bufs=4))

    # ---- loads ----
    xq_sb = big.tile([C, QSH], F32R)
    nc.sync.dma_start(out=xq_sb, in_=xq)
    xkv_sb = big.tile([C, N], F32R)
    nc.sync.dma_start(out=xkv_sb, in_=xkv)
    wqT = const.tile([C, C], F32R)
    nc.sync.dma_start(out=wqT, in_=wq.rearrange("o c -> c o"))
    wkT = const.tile([C, C], F32R)
    nc.sync.dma_start(out=wkT, in_=wk.rearrange("o c -> c o"))
    wvT = const.tile([C, C], F32R)
    nc.sync.dma_start(out=wvT, in_=wv.rearrange("o c -> c o"))
    wpT = const.tile([C, C], F32R)
    nc.sync.dma_start(out=wpT, in_=wp.rearrange("o c -> c o"))
    bq_sb = const.tile([C, 1], F32)
    nc.sync.dma_start(out=bq_sb, in_=bq.rearrange("(c one) -> c one", one=1))
    bk_sb = const.tile([C, 1], F32)
    nc.sync.dma_start(out=bk_sb, in_=bk.rearrange("(c one) -> c one", one=1))
    bv_sb = const.tile([C, 1], F32)
    nc.sync.dma_start(out=bv_sb, in_=bv.rearrange("(c one) -> c one", one=1))
    bp_sb = const.tile([C, 1], F32)
    nc.sync.dma_start(out=bp_sb, in_=bp.rearrange("(c one) -> c one", one=1))
    # memset can't target f32r (it bitcasts internally); stage ones in f32
    ones_f32 = const.tile([128, MCH], F32)
    nc.vector.memset(ones_f32, 1.0)

    # ---- projections ----
    # qT[o, n] on partitions o
    qT_sb = big.tile([C, QSH], BF16)
    for t in range(NT):
        pq = ps.tile([128, 1024], F32, tag="ps")
        nc.tensor.matmul(pq[0:C, 0:512], lhsT=wqT, rhs=xq_sb[:, ts(t, 512)],
                         start=True, stop=True)
        nc.vector.tensor_scalar_add(qT_sb[:, ts(t, 512)], pq[0:C, 0:512], bq_sb)
    # k[o, m] on partitions o
    k_sb = big.tile([C, N], BF16)
    for t in range(N // 512):
        pk = ps.tile([128, 1024], F32, tag="ps")
        nc.tensor.matmul(pk[0:C, 0:512], lhsT=wkT, rhs=xkv_sb[:, ts(t, 512)],
                         start=True, stop=True)
        nc.vector.tensor_scalar_add(k_sb[:, ts(t, 512)], pk[0:C, 0:512], bk_sb)
    # v[m, c] chunks of 128 keys, with an extra ones column (denominator)
    v_sb = big.tile([128, MCH, C + 1], BF16)
    nc.vector.tensor_copy(v_sb[:, :, C], ones_f32)
    for i in range(MCH):
        pv = ps.tile([128, 1024], F32, tag="ps")
        nc.tensor.matmul(pv[:, 0:C], lhsT=xkv_sb[:, ts(i, 128)], rhs=wvT,
                         start=True, stop=True)
        nc.vector.tensor_copy(v_sb[:, i, 0:C], pv[:, 0:C])

    # ---- flash loop over key chunks ----
    ps_acc = ps_acc_p.tile([C + 1, QSH], F32)
    for i in range(MCH):
        for h in range(2):
            s_ps = ps.tile([128, 1024], F32, tag="ps")
            for sub in range(2):
                nt = h * 2 + sub
                nc.tensor.matmul(s_ps[:, ts(sub, 512)],
                                 lhsT=k_sb[:, ts(i, 128)],
                                 rhs=qT_sb[:, ts(nt, 512)],
                                 start=True, stop=True)
            ex = ex_pool.tile([128, 1024], BF16)
            nc.scalar.activation(ex, s_ps, AF.Exp, scale=0.125)
            for sub in range(2):
                nt = h * 2 + sub
                nc.tensor.matmul(ps_acc[:, ts(nt, 512)], lhsT=v_sb[:, i, :],
                                 rhs=ex[:, ts(sub, 512)],
                                 start=(i == 0), stop=(i == MCH - 1),
                                 skip_group_check=True)

    # ---- epilogue: normalize (+bv), project, +bp ----
    recip = small.tile([1, QSH], F32, tag="recip")
    nc.vector.reciprocal(recip, ps_acc[C:C + 1, :])
    rb = small.tile([C, QSH], F32, tag="rb")
    nc.gpsimd.partition_broadcast(rb, recip)
    for t in range(NT):
        pvn = small.tile([C, 512], F32R, tag="pvn")
        nc.vector.tensor_mul(pvn, ps_acc[0:C, ts(t, 512)], rb[:, ts(t, 512)])
        nc.vector.tensor_scalar_add(pvn, pvn, bv_sb)
        pp = ps.tile([128, 1024], F32, tag="ps")
        nc.tensor.matmul(pp[0:C, 0:512], lhsT=wpT, rhs=pvn,
                         start=True, stop=True)
        o_sb = small.tile([C, 512], F32, tag="o")
        nc.scalar.activation(o_sb, pp[0:C, 0:512], AF.Identity, bias=bp_sb,
                             scale=1.0)
        nc.sync.dma_start(out=out[:, ts(t, 512)], in_=o_sb)
    ctx.close()


def _build():
    nc = bacc.Bacc("TRN2", target_bir_lowering=False, debug=False,
                   num_devices=NCORES)
    aps = {}
    aps["xq"] = nc.dram_tensor("xq", [C, QSH], F32R, kind="ExternalInput").ap()
    aps["xkv"] = nc.dram_tensor("xkv", [C, N], F32R, kind="ExternalInput").ap()
    for nm in ("wq", "wk", "wv", "wp"):
        aps[nm] = nc.dram_tensor(nm, [C, C], F32R, kind="ExternalInput").ap()
    for nm in ("bq", "bk", "bv", "bp"):
        aps[nm] = nc.dram_tensor(nm, [C], F32, kind="ExternalInput").ap()
    aps["out"] = nc.dram_tensor("out", [C, QSH], F32, kind="ExternalOutput").ap()
    with tile.TileContext(nc) as tc:
        _emit(tc, **aps)
    nc.finalize()
    return nc


def kernel(branch1, branch2, Wq, bq, Wk, bk, Wv, bv, Wp, bp, **run_kwargs):
    if "nc" not in _CACHE:
        _CACHE["nc"] = _build()
    nc = _CACHE["nc"]

    x1 = np.ascontiguousarray(np.asarray(branch1, np.float32).reshape(B, C, N))
    x2 = np.ascontiguousarray(np.asarray(branch2, np.float32).reshape(B, C, N))
    consts = {
        "wq": np.ascontiguousarray(Wq, np.float32),
        "wk": np.ascontiguousarray(Wk, np.float32),
        "wv": np.ascontiguousarray(Wv, np.float32),
        "wp": np.ascontiguousarray(Wp, np.float32),
        "bq": np.ascontiguousarray(bq, np.float32),
        "bk": np.ascontiguousarray(bk, np.float32),
        "bv": np.ascontiguousarray(bv, np.float32),
        "bp": np.ascontiguousarray(bp, np.float32),
    }
    in_maps = []
    for core in range(NCORES):
        b, s = divmod(core, NCORES // B)
        in_maps.append({
            "xq": np.ascontiguousarray(x1[b, :, s * QSH:(s + 1) * QSH]),
            "xkv": x2[b],
            **consts,
        })
    res = run_bass_kernel_spmd(nc, in_maps, core_ids=list(range(NCORES)),
                               **run_kwargs)
    out = np.empty((B, C, N), np.float32)
    for core in range(NCORES):
        b, s = divmod(core, NCORES // B)
        out[b, :, s * QSH:(s + 1) * QSH] = res.results[core]["out"]
    if run_kwargs:
        _CACHE["last_result"] = res
    return out.reshape(B, C, D, H, W)
